# revision 21
# baseline (speedup 1.0000x reference)
"""CANLayer (GNN message passing) Trainium2 kernel — 8 NeuronCores.

y = sigmoid(L_down @ (x Wc) + L_up @ (x Wc) + x Wl)

Strategy (self-contained: full inputs in, full output out):
  - segment_sum commutes with the dense right-multiplication by Wc, so we
    segment-sum raw x rows per 128-row destination block and apply Wc
    afterward:  s = segsum(val * x[col]);  y = sigmoid(s Wc + x Wl)
  - destination rows are sharded across 8 cores (12500 each).  Both
    Laplacians' COO entries are bucketed by (dest superblock of 4 blocks,
    source quarter, block) on the host; slot padding (caps = max count over
    cores, rounded to 128) keeps the instruction stream identical across
    cores so one SPMD program serves all 8.
  - the source-row gather x[col] is done ON THE HOST (pure permutation,
    np fancy-indexing, no arithmetic): per core a dense fp16 [128, T, 64]
    slot-ordered stream `xg` is staged in HBM and loaded with big
    line-rate HWDGE DMAs — no per-entry SWDGE descriptors.
  - per entry-tile of 128 rows, a scaled one-hot
    S^T[e, r] = val_e * (r == rloc_e) is built on DVE in one fused
    tensor_scalar (is_equal, mult), and the PE accumulates
    s^T[64, 128*SB] += G_t.T @ S_t^T into a single PSUM bank.
  - everything after the segment-sum is f32.
"""
import os

import numpy as np

import concourse.mybir as mybir
import concourse.tile as tile
from concourse import bacc
from concourse import bass_utils

N = 100000
C = 64
NCORES = 8
P = 128                    # entries per tile (matmul contraction dim)
BP = int(os.environ.get("K_BP", "64"))   # dest-block rows (one-hot width)
R = N // NCORES            # 12500 rows per core
NBLK = (R + BP - 1) // BP  # dest blocks per core
RPAD = NBLK * BP
NQ = 4
QROWS = N // NQ            # 25000
SB = 512 // BP             # dest blocks per superblock (one PSUM bank)
NSB = (NBLK + SB - 1) // SB


# one-hot producer assignment: per superblock, repeat [GPB gpsimd-batched
# tiles, DVB dve tiles].  Must match between host prep and device IR.
GPB = int(os.environ.get("K_GPB", "12"))
DVB = int(os.environ.get("K_DVB", "6"))


def _assign(tiles_sb):
    """Per sb: list of ('gp', k) / ('dve', k) runs + per-tile batch pos."""
    runs_sb = []
    pos_sb = []                                 # batch position per tile
    for T_s in tiles_sb:
        runs = []
        pos = []
        t = 0
        while t < int(T_s):
            k = min(GPB, int(T_s) - t)
            runs.append(("gp", k))
            pos.extend(range(k))
            t += k
            if t < int(T_s):
                k = min(DVB, int(T_s) - t)
                runs.append(("dve", k))
                pos.extend([0] * k)
                t += k
        runs_sb.append(runs)
        pos_sb.append(pos)
    return runs_sb, pos_sb


# ---------------------------------------------------------------- host prep

def _preprocess(inputs):
    x = np.ascontiguousarray(np.asarray(inputs["x"], dtype=np.float32))
    w_conv = np.asarray(inputs["w_conv"], dtype=np.float32)
    w_lin = np.asarray(inputs["w_lin"], dtype=np.float32)

    rows = np.concatenate([np.asarray(inputs["down_rows"]),
                           np.asarray(inputs["up_rows"])]).astype(np.int64)
    cols = np.concatenate([np.asarray(inputs["down_cols"]),
                           np.asarray(inputs["up_cols"])]).astype(np.int64)
    vals = np.concatenate([np.asarray(inputs["down_vals"]),
                           np.asarray(inputs["up_vals"])]).astype(np.float32)

    core = rows // R
    rl = rows % R
    blk = rl // BP
    rloc = rl - blk * BP
    q = cols // QROWS

    # group order: (core, superblock, quarter, block-in-superblock)
    sb = blk // SB
    bin_ = blk - sb * SB
    gkey = (sb * NQ + q) * SB + bin_            # within-core group id
    ngpc = NSB * NQ * SB                        # groups per core (incl ghosts)
    key = core * ngpc + gkey
    order = np.argsort(key, kind="stable")
    key_s = key[order]
    col_s = cols[order]
    rloc_s = rloc[order]
    vals_s = vals[order]

    ngroups = NCORES * ngpc
    counts = np.bincount(key_s, minlength=ngroups).reshape(NCORES, ngpc)
    caps = counts.max(axis=0)                   # [ngpc]
    # ghost groups (blocks beyond NBLK in the last superblock) stay size 0
    g_ids = np.arange(ngpc)
    g_blk = (g_ids // (NQ * SB)) * SB + (g_ids % SB)
    ghost = g_blk >= NBLK
    caps = np.where(ghost, 0, np.maximum(((caps + P - 1) // P) * P, P))
    S_total = int(caps.sum())
    T_total = S_total // P

    group_off = np.zeros(ngpc, dtype=np.int64)
    group_off[1:] = np.cumsum(caps)[:-1]

    starts = np.zeros(ngroups + 1, dtype=np.int64)
    starts[1:] = np.cumsum(counts.reshape(-1))
    within = np.arange(len(key_s)) - starts[key_s]
    slot = group_off[key_s % ngpc] + within
    ecore = key_s // ngpc

    col_pad = np.zeros((NCORES, S_total), dtype=np.int64)
    rloc_pad = np.zeros((NCORES, S_total), dtype=np.float32)
    val_pad = np.zeros((NCORES, S_total), dtype=np.float32)
    col_pad[ecore, slot] = col_s
    rloc_pad[ecore, slot] = rloc_s.astype(np.float32)
    val_pad[ecore, slot] = vals_s
    # padding slots keep col=0, rloc=0, val=0 -> zero contribution
    # NB: pad slots scatter val=0 to rloc=0 (harmless; duplicates with a
    # real rloc=0 entry in the same partition are fine for the DVE one-hot
    # but local_scatter forbids dup idxs -- it doesn't: dups are across
    # idx columns of one call, and we pass a single real idx per call.

    x16 = x.astype(np.float16)                  # [N, 64]
    wcwl = np.concatenate([w_conv, w_lin], axis=1)  # [64, 128] f32

    in_maps = []
    for c in range(NCORES):
        # host-side gather: dense slot-ordered source rows, [128, T, 64]
        xg = np.ascontiguousarray(
            x16[col_pad[c]].reshape(T_total, P, C).transpose(1, 0, 2))
        rv = np.empty((P, 2 * T_total), dtype=np.float32)
        rv[:, 0::2] = rloc_pad[c].reshape(T_total, P).T
        rv[:, 1::2] = val_pad[c].reshape(T_total, P).T
        # local_scatter metadata: idx pairs (rloc + 128*batch_pos, -1),
        # data pairs (val, 0); batch offsets follow the _assign pattern
        tiles_sb_h = caps.reshape(NSB, NQ * SB).sum(axis=1) // P
        _, pos_sb = _assign(tiles_sb_h)
        bpos = np.concatenate([np.asarray(p, dtype=np.int64)
                               for p in pos_sb if len(p)])
        ri = np.full((P, 2 * T_total), -1, dtype=np.int16)
        ri[:, 0::2] = (rloc_pad[c].reshape(T_total, P).T
                       + (bpos * BP)[None, :]).astype(np.int16)
        vh = np.zeros((P, 2 * T_total), dtype=np.float16)
        vh[:, 0::2] = val_pad[c].reshape(T_total, P).T.astype(np.float16)
        xT = np.zeros((C, RPAD), dtype=np.float32)
        xT[:, :R] = x[c * R:(c + 1) * R].T
        in_maps.append({
            "xg": xg,
            "rv": np.ascontiguousarray(rv),
            "ri": np.ascontiguousarray(ri),
            "vh": np.ascontiguousarray(vh),
            "xt": xT,
            "w": np.ascontiguousarray(wcwl),
        })
    return in_maps, caps.reshape(NSB, NQ, SB)


# ---------------------------------------------------------------- device IR

def _build(caps, nsb_limit=None):
    caps = np.asarray(caps)                     # [NSB, NQ, SB]

    nsb = int(nsb_limit or os.environ.get("K_NSB", NSB))
    S_total = int(caps.sum())
    T_total = S_total // P
    tiles_sb = caps.sum(axis=(1, 2)) // P       # tiles per superblock
    T_max = int(tiles_sb.max())
    OGRP = 1024 // BP  # output blocks staged per out DMA

    nc = bacc.Bacc("TRN2", target_bir_lowering=False, debug=False,
                   enable_asserts=False, num_devices=NCORES)
    xg_d = nc.dram_tensor("xg", [P, T_total, C], mybir.dt.float16,
                          kind="ExternalInput").ap()
    rv_d = nc.dram_tensor("rv", [P, 2 * T_total], mybir.dt.float32,
                          kind="ExternalInput").ap()
    ri_d = nc.dram_tensor("ri", [P, 2 * T_total], mybir.dt.int16,
                          kind="ExternalInput").ap()
    vh_d = nc.dram_tensor("vh", [P, 2 * T_total], mybir.dt.float16,
                          kind="ExternalInput").ap()
    xt_d = nc.dram_tensor("xt", [C, RPAD], mybir.dt.float32,
                          kind="ExternalInput").ap()
    w_d = nc.dram_tensor("w", [C, 2 * C], mybir.dt.float32,
                         kind="ExternalInput").ap()
    out_d = nc.dram_tensor("out", [BP, NBLK, C], mybir.dt.float32,
                           kind="ExternalOutput").ap()

    with tile.TileContext(nc) as tc:
        with tc.tile_pool(name="const", bufs=1) as cpool, \
             tc.tile_pool(name="gb", bufs=3) as gpool, \
             tc.tile_pool(name="meta", bufs=3) as mpool, \
             tc.tile_pool(name="oh", bufs=8) as ohpool, \
             tc.tile_pool(name="stg", bufs=2) as spool, \
             tc.tile_pool(name="ps1", bufs=2, space="PSUM") as ps1, \
             tc.tile_pool(name="ps2", bufs=2, space="PSUM") as ps2:

            # constants
            iota_i = cpool.tile([P, BP], mybir.dt.int16)
            nc.gpsimd.iota(iota_i[:], pattern=[[1, BP]], base=0,
                           channel_multiplier=0)
            iota_f = cpool.tile([P, BP], mybir.dt.float16)
            nc.vector.tensor_copy(iota_f[:], iota_i[:])
            w_t = cpool.tile([C, 2 * C], mybir.dt.float32)
            nc.sync.dma_start(w_t[:], w_d)
            xt_t = cpool.tile([C, RPAD], mybir.dt.float32)
            nc.sync.dma_start(xt_t[:], xt_d)

            reps = int(os.environ.get("K_REPS", "1"))
            import contextlib
            rep_ctx = tc.For_i(0, reps, 1) if reps > 1 else \
                contextlib.nullcontext()
            with rep_ctx:
                self_body(nc, tc, caps, nsb, tiles_sb, T_max, OGRP,
                          iota_f, w_t, xt_t, gpool, mpool, ohpool, spool,
                          ps1, ps2, xg_d, rv_d, ri_d, vh_d, out_d)
    nc.compile()
    return nc


def self_body(nc, tc, caps, nsb, tiles_sb, T_max, OGRP,
              iota_f, w_t, xt_t, gpool, mpool, ohpool, spool,
              ps1, ps2, xg_d, rv_d, ri_d, vh_d, out_d):
    runs_sb, _ = _assign(tiles_sb)
    tile_off = 0   # entry-tiles consumed so far
    ob = None

    def stage2(s, psum_sT):
        nonlocal ob
        k_sb = min(SB, NBLK - s * SB)
        for bb in range(k_sb):
            b = s * SB + bb
            sT_sb = spool.tile([C, BP], mybir.dt.float32, tag="sT")
            nc.scalar.copy(sT_sb[:], psum_sT[:, bb * BP:(bb + 1) * BP])

            out2 = ps2.tile([BP, C], mybir.dt.float32)
            nc.tensor.matmul(out2[:], sT_sb[:], w_t[:, 0:C],
                             start=True, stop=False)
            nc.tensor.matmul(out2[:], xt_t[:, b * BP:(b + 1) * BP],
                             w_t[:, C:2 * C], start=False, stop=True)

            g = b // OGRP
            j = b % OGRP
            gsz = min(OGRP, NBLK - g * OGRP)
            if j == 0:
                ob = spool.tile([BP, OGRP, C], mybir.dt.float32,
                                tag="ob")
            nc.scalar.activation(ob[:, j, :], out2[:],
                                 mybir.ActivationFunctionType.Sigmoid)
            if j == gsz - 1:
                nc.sync.dma_start(
                    out_d[:, g * OGRP:g * OGRP + gsz, :],
                    ob[:, :gsz, :])

    pending = None   # (s, psum_sT) whose per-block stage runs one sb late
    for s in range(nsb):
        T_s = int(tiles_sb[s])

        rv_t = mpool.tile([P, 2 * T_max], mybir.dt.float32, tag="rv")
        nc.sync.dma_start(
            rv_t[:, :2 * T_s],
            rv_d[:, 2 * tile_off: 2 * (tile_off + T_s)])
        ri_t = mpool.tile([P, 2 * T_max], mybir.dt.int16, tag="ri")
        nc.sync.dma_start(
            ri_t[:, :2 * T_s],
            ri_d[:, 2 * tile_off: 2 * (tile_off + T_s)])
        vh_t = mpool.tile([P, 2 * T_max], mybir.dt.float16, tag="vh")
        nc.sync.dma_start(
            vh_t[:, :2 * T_s],
            vh_d[:, 2 * tile_off: 2 * (tile_off + T_s)])

        gbuf = gpool.tile([P, T_max, C], mybir.dt.float16, tag="g")
        nc.sync.dma_start(
            gbuf[:, :T_s, :],
            xg_d[:, tile_off:tile_off + T_s, :])

        # segment-sum all tiles into one PSUM bank [64, SB*BP]
        psum_sT = ps1.tile([C, SB * BP], mybir.dt.float32)
        tile_blocks = []
        for qq in range(NQ):
            for bb in range(SB):
                tile_blocks += [bb] * (int(caps[s, qq, bb]) // P)
        T_sb = len(tile_blocks)

        def mm(t, st_ap):
            # start=True zeroes the whole 2KB zero-region (= this
            # bank), initializing every block's 128-col span at once;
            # one accumulation group covers the whole superblock.
            nc.tensor.matmul(
                psum_sT[:, tile_blocks[t] * BP:(tile_blocks[t] + 1) * BP],
                gbuf[:, t, :], st_ap,
                start=(t == 0),
                stop=(t == T_sb - 1),
            )

        t = 0
        for kind, k in runs_sb[s]:
            if kind == "gp":
                # k-tile batched scaled one-hot via GPSIMD local scatter:
                # stb[:]=0; stb[p, rloc[p,j] + 128*j] = val[p,j]
                # (odd idxs are -1 -> ignored)
                stb = ohpool.tile([P, GPB * BP], mybir.dt.float16, tag="gob")
                nc.gpsimd.local_scatter(
                    stb[:, :k * BP],
                    vh_t[:, 2 * t:2 * (t + k)],
                    ri_t[:, 2 * t:2 * (t + k)],
                    channels=P, num_elems=k * BP, num_idxs=2 * k,
                )
                for j in range(k):
                    mm(t + j, stb[:, j * BP:(j + 1) * BP])
            else:
                for j in range(k):
                    st = ohpool.tile([P, BP], mybir.dt.float16, tag="oh")
                    nc.vector.tensor_scalar(
                        out=st[:],
                        in0=iota_f[:],
                        scalar1=rv_t[:, 2 * (t + j):2 * (t + j) + 1],
                        scalar2=rv_t[:, 2 * (t + j) + 1:2 * (t + j) + 2],
                        op0=mybir.AluOpType.is_equal,
                        op1=mybir.AluOpType.mult,
                    )
                    mm(t + j, st[:])
            t += k

        if pending is not None:
            stage2(*pending)
        pending = (s, psum_sT)

        tile_off += T_s

    if pending is not None:
        stage2(*pending)


# ---------------------------------------------------------------- entry

_CACHE = {}


def _prepare(inputs):
    in_maps, caps = _preprocess(inputs)
    key = caps.tobytes()
    if key not in _CACHE:
        _CACHE[key] = _build(caps)
    return _CACHE[key], in_maps


def kernel(**inputs):
    nc, in_maps = _prepare(inputs)
    res = bass_utils.run_bass_kernel_spmd(nc, in_maps,
                                          core_ids=list(range(NCORES)))
    outs = []
    for c in range(NCORES):
        o = res.results[c]["out"]          # [BP, NBLK, C]
        outs.append(o.transpose(1, 0, 2).reshape(RPAD, C)[:R])
    return np.concatenate(outs, axis=0).astype(np.float32)


# revision 25
# speedup vs baseline: 1.1253x; 1.1253x over previous
"""CANLayer (GNN message passing) Trainium2 kernel — 8 NeuronCores.

y = sigmoid(L_down @ (x Wc) + L_up @ (x Wc) + x Wl)

Strategy (self-contained: full inputs in, full output out):
  - segment_sum commutes with the dense right-multiplication by Wc, so we
    segment-sum raw x rows per 128-row destination block and apply Wc
    afterward:  s = segsum(val * x[col]);  y = sigmoid(s Wc + x Wl)
  - destination rows are sharded across 8 cores (12500 each).  Both
    Laplacians' COO entries are bucketed by (dest superblock of 4 blocks,
    source quarter, block) on the host; slot padding (caps = max count over
    cores, rounded to 128) keeps the instruction stream identical across
    cores so one SPMD program serves all 8.
  - the source-row gather x[col] is done ON THE HOST (pure permutation,
    np fancy-indexing, no arithmetic): per core a dense fp16 [128, T, 64]
    slot-ordered stream `xg` is staged in HBM and loaded with big
    line-rate HWDGE DMAs — no per-entry SWDGE descriptors.
  - per entry-tile of 128 rows, a scaled one-hot
    S^T[e, r] = val_e * (r == rloc_e) is built on DVE in one fused
    tensor_scalar (is_equal, mult), and the PE accumulates
    s^T[64, 128*SB] += G_t.T @ S_t^T into a single PSUM bank.
  - everything after the segment-sum is f32.
"""
import os

import numpy as np

import concourse.mybir as mybir
import concourse.tile as tile
from concourse import bacc
from concourse import bass_utils

N = 100000
C = 64
NCORES = 8
P = 128                    # entries per tile (matmul contraction dim)
BP = int(os.environ.get("K_BP", "64"))   # dest-block rows (one-hot width)
R = N // NCORES            # 12500 rows per core
NBLK = (R + BP - 1) // BP  # dest blocks per core
RPAD = NBLK * BP
NQ = 4
QROWS = N // NQ            # 25000
SB = 512 // BP             # dest blocks per superblock (one PSUM bank)
NSB = (NBLK + SB - 1) // SB


# one-hot producer assignment: per superblock, repeat [GPB gpsimd-batched
# tiles, DVB dve tiles].  Must match between host prep and device IR.
GPB = int(os.environ.get("K_GPB", "12"))
DVB = int(os.environ.get("K_DVB", "4"))


def _assign(tiles_sb):
    """Per sb: list of ('gp', k) / ('dve', k) runs + per-tile batch pos."""
    runs_sb = []
    pos_sb = []                                 # batch position per tile
    for T_s in tiles_sb:
        runs = []
        pos = []
        t = 0
        while t < int(T_s):
            k = min(GPB, int(T_s) - t)
            runs.append(("gp", k))
            pos.extend(range(k))
            t += k
            if t < int(T_s):
                k = min(DVB, int(T_s) - t)
                runs.append(("dve", k))
                pos.extend([0] * k)
                t += k
        runs_sb.append(runs)
        pos_sb.append(pos)
    return runs_sb, pos_sb


# ---------------------------------------------------------------- host prep

def _preprocess(inputs):
    x = np.ascontiguousarray(np.asarray(inputs["x"], dtype=np.float32))
    w_conv = np.asarray(inputs["w_conv"], dtype=np.float32)
    w_lin = np.asarray(inputs["w_lin"], dtype=np.float32)

    rows = np.concatenate([np.asarray(inputs["down_rows"]),
                           np.asarray(inputs["up_rows"])]).astype(np.int64)
    cols = np.concatenate([np.asarray(inputs["down_cols"]),
                           np.asarray(inputs["up_cols"])]).astype(np.int64)
    vals = np.concatenate([np.asarray(inputs["down_vals"]),
                           np.asarray(inputs["up_vals"])]).astype(np.float32)

    core = rows // R
    rl = rows % R
    blk = rl // BP
    rloc = rl - blk * BP
    q = cols // QROWS

    # group order: (core, superblock, quarter, block-in-superblock)
    sb = blk // SB
    bin_ = blk - sb * SB
    gkey = (sb * NQ + q) * SB + bin_            # within-core group id
    ngpc = NSB * NQ * SB                        # groups per core (incl ghosts)
    key = core * ngpc + gkey
    order = np.argsort(key, kind="stable")
    key_s = key[order]
    col_s = cols[order]
    rloc_s = rloc[order]
    vals_s = vals[order]

    ngroups = NCORES * ngpc
    counts = np.bincount(key_s, minlength=ngroups).reshape(NCORES, ngpc)
    caps = counts.max(axis=0)                   # [ngpc]
    # ghost groups (blocks beyond NBLK in the last superblock) stay size 0
    g_ids = np.arange(ngpc)
    g_blk = (g_ids // (NQ * SB)) * SB + (g_ids % SB)
    ghost = g_blk >= NBLK
    caps = np.where(ghost, 0, np.maximum(((caps + P - 1) // P) * P, P))
    S_total = int(caps.sum())
    T_total = S_total // P

    group_off = np.zeros(ngpc, dtype=np.int64)
    group_off[1:] = np.cumsum(caps)[:-1]

    starts = np.zeros(ngroups + 1, dtype=np.int64)
    starts[1:] = np.cumsum(counts.reshape(-1))
    within = np.arange(len(key_s)) - starts[key_s]
    slot = group_off[key_s % ngpc] + within
    ecore = key_s // ngpc

    col_pad = np.zeros((NCORES, S_total), dtype=np.int64)
    rloc_pad = np.zeros((NCORES, S_total), dtype=np.float32)
    val_pad = np.zeros((NCORES, S_total), dtype=np.float32)
    col_pad[ecore, slot] = col_s
    rloc_pad[ecore, slot] = rloc_s.astype(np.float32)
    val_pad[ecore, slot] = vals_s
    # padding slots keep col=0, rloc=0, val=0 -> zero contribution
    # NB: pad slots scatter val=0 to rloc=0 (harmless; duplicates with a
    # real rloc=0 entry in the same partition are fine for the DVE one-hot
    # but local_scatter forbids dup idxs -- it doesn't: dups are across
    # idx columns of one call, and we pass a single real idx per call.

    x16 = x.astype(np.float16)                  # [N, 64]
    wcwl = np.concatenate([w_conv, w_lin], axis=1)  # [64, 128] f32

    in_maps = []
    for c in range(NCORES):
        # host-side gather: dense slot-ordered source rows, [128, T, 64]
        xg = np.ascontiguousarray(
            x16[col_pad[c]].reshape(T_total, P, C).transpose(1, 0, 2))
        rv = np.empty((P, 2 * T_total), dtype=np.float32)
        rv[:, 0::2] = rloc_pad[c].reshape(T_total, P).T
        rv[:, 1::2] = val_pad[c].reshape(T_total, P).T
        # local_scatter metadata: idx pairs (rloc + 128*batch_pos, -1),
        # data pairs (val, 0); batch offsets follow the _assign pattern
        tiles_sb_h = caps.reshape(NSB, NQ * SB).sum(axis=1) // P
        _, pos_sb = _assign(tiles_sb_h)
        bpos = np.concatenate([np.asarray(p, dtype=np.int64)
                               for p in pos_sb if len(p)])
        ri = np.full((P, 2 * T_total), -1, dtype=np.int16)
        ri[:, 0::2] = (rloc_pad[c].reshape(T_total, P).T
                       + (bpos * BP)[None, :]).astype(np.int16)
        vh = np.zeros((P, 2 * T_total), dtype=np.float16)
        vh[:, 0::2] = val_pad[c].reshape(T_total, P).T.astype(np.float16)
        xT = np.zeros((C, RPAD), dtype=np.float32)
        xT[:, :R] = x[c * R:(c + 1) * R].T
        in_maps.append({
            "xg": xg,
            "rv": np.ascontiguousarray(rv),
            "ri": np.ascontiguousarray(ri),
            "vh": np.ascontiguousarray(vh),
            "xt": xT,
            "w": np.ascontiguousarray(wcwl),
        })
    return in_maps, caps.reshape(NSB, NQ, SB)


# ---------------------------------------------------------------- device IR

def _build(caps, nsb_limit=None):
    caps = np.asarray(caps)                     # [NSB, NQ, SB]

    nsb = int(nsb_limit or os.environ.get("K_NSB", NSB))
    S_total = int(caps.sum())
    T_total = S_total // P
    tiles_sb = caps.sum(axis=(1, 2)) // P       # tiles per superblock
    T_max = int(tiles_sb.max())
    OGRP = 8   # output row-pairs (128 rows each) staged per out DMA

    nc = bacc.Bacc("TRN2", target_bir_lowering=False, debug=False,
                   enable_asserts=False, num_devices=NCORES)
    xg_d = nc.dram_tensor("xg", [P, T_total, C], mybir.dt.float16,
                          kind="ExternalInput").ap()
    rv_d = nc.dram_tensor("rv", [P, 2 * T_total], mybir.dt.float32,
                          kind="ExternalInput").ap()
    ri_d = nc.dram_tensor("ri", [P, 2 * T_total], mybir.dt.int16,
                          kind="ExternalInput").ap()
    vh_d = nc.dram_tensor("vh", [P, 2 * T_total], mybir.dt.float16,
                          kind="ExternalInput").ap()
    xt_d = nc.dram_tensor("xt", [C, RPAD], mybir.dt.float32,
                          kind="ExternalInput").ap()
    w_d = nc.dram_tensor("w", [C, 2 * C], mybir.dt.float32,
                         kind="ExternalInput").ap()
    out_d = nc.dram_tensor("out", [128, RPAD // 128, C], mybir.dt.float32,
                           kind="ExternalOutput").ap()

    with tile.TileContext(nc) as tc:
        with tc.tile_pool(name="const", bufs=1) as cpool, \
             tc.tile_pool(name="gb", bufs=3) as gpool, \
             tc.tile_pool(name="meta", bufs=3) as mpool, \
             tc.tile_pool(name="oh", bufs=8) as ohpool, \
             tc.tile_pool(name="stg", bufs=2) as spool, \
             tc.tile_pool(name="ps1", bufs=2, space="PSUM") as ps1, \
             tc.tile_pool(name="ps2", bufs=2, space="PSUM") as ps2:

            # constants
            iota_i = cpool.tile([P, BP], mybir.dt.int16)
            nc.gpsimd.iota(iota_i[:], pattern=[[1, BP]], base=0,
                           channel_multiplier=0)
            iota_f = cpool.tile([P, BP], mybir.dt.float16)
            nc.vector.tensor_copy(iota_f[:], iota_i[:])
            w_t = cpool.tile([C, 2 * C], mybir.dt.float32)
            nc.sync.dma_start(w_t[:], w_d)
            xt_t = cpool.tile([C, RPAD], mybir.dt.float32)
            nc.sync.dma_start(xt_t[:], xt_d)

            reps = int(os.environ.get("K_REPS", "1"))
            import contextlib
            rep_ctx = tc.For_i(0, reps, 1) if reps > 1 else \
                contextlib.nullcontext()
            with rep_ctx:
                self_body(nc, tc, caps, nsb, tiles_sb, T_max, OGRP,
                          iota_f, w_t, xt_t, gpool, mpool, ohpool, spool,
                          ps1, ps2, xg_d, rv_d, ri_d, vh_d, out_d)
    nc.compile()
    return nc


def self_body(nc, tc, caps, nsb, tiles_sb, T_max, OGRP,
              iota_f, w_t, xt_t, gpool, mpool, ohpool, spool,
              ps1, ps2, xg_d, rv_d, ri_d, vh_d, out_d):
    runs_sb, _ = _assign(tiles_sb)
    tile_off = 0   # entry-tiles consumed so far
    ob = None

    PW = 128                         # output row-pair width
    NPAIR = RPAD // PW

    def stage2(s, psum_sT):
        # one wide PSUM->SBUF copy per superblock, then 128-row pair
        # weight matmuls + sigmoid + staged output DMA
        nonlocal ob
        k_sb = min(SB, NBLK - s * SB)
        rows = k_sb * BP
        sT_buf = spool.tile([C, SB * BP], mybir.dt.float32, tag="sT")
        nc.scalar.copy(sT_buf[:, :rows], psum_sT[:, :rows])
        for j2 in range(rows // PW):
            b2 = (s * SB * BP) // PW + j2
            out2 = ps2.tile([PW, C], mybir.dt.float32)
            nc.tensor.matmul(out2[:], sT_buf[:, j2 * PW:(j2 + 1) * PW],
                             w_t[:, 0:C], start=True, stop=False)
            nc.tensor.matmul(out2[:], xt_t[:, b2 * PW:(b2 + 1) * PW],
                             w_t[:, C:2 * C], start=False, stop=True)

            g = b2 // OGRP
            j = b2 % OGRP
            gsz = min(OGRP, NPAIR - g * OGRP)
            if j == 0:
                ob = spool.tile([PW, OGRP, C], mybir.dt.float32,
                                tag="ob")
            nc.scalar.activation(ob[:, j, :], out2[:],
                                 mybir.ActivationFunctionType.Sigmoid)
            if j == gsz - 1:
                nc.sync.dma_start(
                    out_d[:, g * OGRP:g * OGRP + gsz, :],
                    ob[:, :gsz, :])

    pending = None   # (s, psum_sT) whose per-block stage runs one sb late
    for s in range(nsb):
        T_s = int(tiles_sb[s])

        rv_t = mpool.tile([P, 2 * T_max], mybir.dt.float32, tag="rv")
        nc.sync.dma_start(
            rv_t[:, :2 * T_s],
            rv_d[:, 2 * tile_off: 2 * (tile_off + T_s)])
        ri_t = mpool.tile([P, 2 * T_max], mybir.dt.int16, tag="ri")
        nc.sync.dma_start(
            ri_t[:, :2 * T_s],
            ri_d[:, 2 * tile_off: 2 * (tile_off + T_s)])
        vh_t = mpool.tile([P, 2 * T_max], mybir.dt.float16, tag="vh")
        nc.sync.dma_start(
            vh_t[:, :2 * T_s],
            vh_d[:, 2 * tile_off: 2 * (tile_off + T_s)])

        gbuf = gpool.tile([P, T_max, C], mybir.dt.float16, tag="g")
        nc.sync.dma_start(
            gbuf[:, :T_s, :],
            xg_d[:, tile_off:tile_off + T_s, :])

        # segment-sum all tiles into one PSUM bank [64, SB*BP]
        psum_sT = ps1.tile([C, SB * BP], mybir.dt.float32)
        tile_blocks = []
        for qq in range(NQ):
            for bb in range(SB):
                tile_blocks += [bb] * (int(caps[s, qq, bb]) // P)
        T_sb = len(tile_blocks)

        def mm(t, st_ap):
            # start=True zeroes the whole 2KB zero-region (= this
            # bank), initializing every block's 128-col span at once;
            # one accumulation group covers the whole superblock.
            nc.tensor.matmul(
                psum_sT[:, tile_blocks[t] * BP:(tile_blocks[t] + 1) * BP],
                gbuf[:, t, :], st_ap,
                start=(t == 0),
                stop=(t == T_sb - 1),
            )

        t = 0
        for kind, k in runs_sb[s]:
            if kind == "gp":
                # k-tile batched scaled one-hot via GPSIMD local scatter:
                # stb[:]=0; stb[p, rloc[p,j] + 128*j] = val[p,j]
                # (odd idxs are -1 -> ignored)
                stb = ohpool.tile([P, GPB * BP], mybir.dt.float16, tag="gob")
                nc.gpsimd.local_scatter(
                    stb[:, :k * BP],
                    vh_t[:, 2 * t:2 * (t + k)],
                    ri_t[:, 2 * t:2 * (t + k)],
                    channels=P, num_elems=k * BP, num_idxs=2 * k,
                )
                for j in range(k):
                    mm(t + j, stb[:, j * BP:(j + 1) * BP])
            else:
                for j in range(k):
                    st = ohpool.tile([P, BP], mybir.dt.float16, tag="oh")
                    nc.vector.tensor_scalar(
                        out=st[:],
                        in0=iota_f[:],
                        scalar1=rv_t[:, 2 * (t + j):2 * (t + j) + 1],
                        scalar2=rv_t[:, 2 * (t + j) + 1:2 * (t + j) + 2],
                        op0=mybir.AluOpType.is_equal,
                        op1=mybir.AluOpType.mult,
                    )
                    mm(t + j, st[:])
            t += k

        if pending is not None:
            stage2(*pending)
        pending = (s, psum_sT)

        tile_off += T_s

    if pending is not None:
        stage2(*pending)


# ---------------------------------------------------------------- entry

_CACHE = {}


def _prepare(inputs):
    in_maps, caps = _preprocess(inputs)
    key = caps.tobytes()
    if key not in _CACHE:
        _CACHE[key] = _build(caps)
    return _CACHE[key], in_maps


def kernel(**inputs):
    nc, in_maps = _prepare(inputs)
    res = bass_utils.run_bass_kernel_spmd(nc, in_maps,
                                          core_ids=list(range(NCORES)))
    outs = []
    for c in range(NCORES):
        o = res.results[c]["out"]          # [BP, NBLK, C]
        outs.append(o.transpose(1, 0, 2).reshape(RPAD, C)[:R])
    return np.concatenate(outs, axis=0).astype(np.float32)


# revision 27
# speedup vs baseline: 1.2488x; 1.1098x over previous
"""CANLayer (GNN message passing) Trainium2 kernel — 8 NeuronCores.

y = sigmoid(L_down @ (x Wc) + L_up @ (x Wc) + x Wl)

Strategy (self-contained: full inputs in, full output out):
  - segment_sum commutes with the dense right-multiplication by Wc, so we
    segment-sum raw x rows per 128-row destination block and apply Wc
    afterward:  s = segsum(val * x[col]);  y = sigmoid(s Wc + x Wl)
  - destination rows are sharded across 8 cores (12500 each).  Both
    Laplacians' COO entries are bucketed by (dest superblock of 4 blocks,
    source quarter, block) on the host; slot padding (caps = max count over
    cores, rounded to 128) keeps the instruction stream identical across
    cores so one SPMD program serves all 8.
  - the source-row gather x[col] is done ON THE HOST (pure permutation,
    np fancy-indexing, no arithmetic): per core a dense fp16 [128, T, 64]
    slot-ordered stream `xg` is staged in HBM and loaded with big
    line-rate HWDGE DMAs — no per-entry SWDGE descriptors.
  - per entry-tile of 128 rows, a scaled one-hot
    S^T[e, r] = val_e * (r == rloc_e) is built on DVE in one fused
    tensor_scalar (is_equal, mult), and the PE accumulates
    s^T[64, 128*SB] += G_t.T @ S_t^T into a single PSUM bank.
  - everything after the segment-sum is f32.
"""
import os

import numpy as np

import concourse.mybir as mybir
import concourse.tile as tile
from concourse import bacc
from concourse import bass_utils

N = 100000
C = 64
NCORES = 8
P = 128                    # entries per tile (matmul contraction dim)
BP = int(os.environ.get("K_BP", "64"))   # dest-block rows (one-hot width)
R = N // NCORES            # 12500 rows per core
NBLK = (R + BP - 1) // BP  # dest blocks per core
RPAD = NBLK * BP
NQ = 4
QROWS = N // NQ            # 25000
SB = 512 // BP             # dest blocks per superblock (one PSUM bank)
NSB = (NBLK + SB - 1) // SB


# one-hot producer assignment: per superblock, repeat [GPB gpsimd-batched
# tiles, DVB dve tiles].  Must match between host prep and device IR.
GPB = int(os.environ.get("K_GPB", "12"))
DVB = int(os.environ.get("K_DVB", "4"))


def _assign(tiles_sb):
    """Per sb: list of ('gp', k) / ('dve', k) runs + per-tile batch pos."""
    runs_sb = []
    pos_sb = []                                 # batch position per tile
    for T_s in tiles_sb:
        runs = []
        pos = []
        t = 0
        while t < int(T_s):
            k = min(GPB, int(T_s) - t)
            runs.append(("gp", k))
            pos.extend(range(k))
            t += k
            if t < int(T_s):
                k = min(DVB, int(T_s) - t)
                runs.append(("dve", k))
                pos.extend([0] * k)
                t += k
        runs_sb.append(runs)
        pos_sb.append(pos)
    return runs_sb, pos_sb


# ---------------------------------------------------------------- host prep

def _preprocess(inputs):
    x = np.ascontiguousarray(np.asarray(inputs["x"], dtype=np.float32))
    w_conv = np.asarray(inputs["w_conv"], dtype=np.float32)
    w_lin = np.asarray(inputs["w_lin"], dtype=np.float32)

    rows = np.concatenate([np.asarray(inputs["down_rows"]),
                           np.asarray(inputs["up_rows"])]).astype(np.int64)
    cols = np.concatenate([np.asarray(inputs["down_cols"]),
                           np.asarray(inputs["up_cols"])]).astype(np.int64)
    vals = np.concatenate([np.asarray(inputs["down_vals"]),
                           np.asarray(inputs["up_vals"])]).astype(np.float32)

    core = rows // R
    rl = rows % R
    blk = rl // BP
    rloc = rl - blk * BP
    q = cols // QROWS

    # group order: (core, superblock, quarter, block-in-superblock)
    sb = blk // SB
    bin_ = blk - sb * SB
    gkey = (sb * NQ + q) * SB + bin_            # within-core group id
    ngpc = NSB * NQ * SB                        # groups per core (incl ghosts)
    key = core * ngpc + gkey
    order = np.argsort(key, kind="stable")
    key_s = key[order]
    col_s = cols[order]
    rloc_s = rloc[order]
    vals_s = vals[order]

    ngroups = NCORES * ngpc
    counts = np.bincount(key_s, minlength=ngroups).reshape(NCORES, ngpc)
    caps = counts.max(axis=0)                   # [ngpc]
    # ghost groups (blocks beyond NBLK in the last superblock) stay size 0
    g_ids = np.arange(ngpc)
    g_blk = (g_ids // (NQ * SB)) * SB + (g_ids % SB)
    ghost = g_blk >= NBLK
    caps = np.where(ghost, 0, np.maximum(((caps + P - 1) // P) * P, P))
    S_total = int(caps.sum())
    T_total = S_total // P

    group_off = np.zeros(ngpc, dtype=np.int64)
    group_off[1:] = np.cumsum(caps)[:-1]

    starts = np.zeros(ngroups + 1, dtype=np.int64)
    starts[1:] = np.cumsum(counts.reshape(-1))
    within = np.arange(len(key_s)) - starts[key_s]
    slot = group_off[key_s % ngpc] + within
    ecore = key_s // ngpc

    col_pad = np.zeros((NCORES, S_total), dtype=np.int64)
    rloc_pad = np.zeros((NCORES, S_total), dtype=np.float32)
    val_pad = np.zeros((NCORES, S_total), dtype=np.float32)
    col_pad[ecore, slot] = col_s
    rloc_pad[ecore, slot] = rloc_s.astype(np.float32)
    val_pad[ecore, slot] = vals_s
    # padding slots keep col=0, rloc=0, val=0 -> zero contribution
    # NB: pad slots scatter val=0 to rloc=0 (harmless; duplicates with a
    # real rloc=0 entry in the same partition are fine for the DVE one-hot
    # but local_scatter forbids dup idxs -- it doesn't: dups are across
    # idx columns of one call, and we pass a single real idx per call.

    x16 = x.astype(np.float16)                  # [N, 64]
    wcwl = np.concatenate([w_conv, w_lin], axis=1)  # [64, 128] f32

    in_maps = []
    for c in range(NCORES):
        # host-side gather: dense slot-ordered source rows, [128, T, 64]
        xg = np.ascontiguousarray(
            x16[col_pad[c]].reshape(T_total, P, C).transpose(1, 0, 2))
        rv = np.empty((P, 2 * T_total), dtype=np.float32)
        rv[:, 0::2] = rloc_pad[c].reshape(T_total, P).T
        rv[:, 1::2] = val_pad[c].reshape(T_total, P).T
        # local_scatter metadata: idx pairs (rloc + 128*batch_pos, -1),
        # data pairs (val, 0); batch offsets follow the _assign pattern
        tiles_sb_h = caps.reshape(NSB, NQ * SB).sum(axis=1) // P
        _, pos_sb = _assign(tiles_sb_h)
        bpos = np.concatenate([np.asarray(p, dtype=np.int64)
                               for p in pos_sb if len(p)])
        ri = np.full((P, 2 * T_total), -1, dtype=np.int16)
        ri[:, 0::2] = (rloc_pad[c].reshape(T_total, P).T
                       + (bpos * BP)[None, :]).astype(np.int16)
        vh = np.zeros((P, 2 * T_total), dtype=np.float16)
        vh[:, 0::2] = val_pad[c].reshape(T_total, P).T.astype(np.float16)
        xT = np.zeros((C, RPAD), dtype=np.float32)
        xT[:, :R] = x[c * R:(c + 1) * R].T
        in_maps.append({
            "xg": xg,
            "rv": np.ascontiguousarray(rv),
            "ri": np.ascontiguousarray(ri),
            "vh": np.ascontiguousarray(vh),
            "xt": xT,
            "w": np.ascontiguousarray(wcwl),
        })
    return in_maps, caps.reshape(NSB, NQ, SB)


# ---------------------------------------------------------------- device IR

def _build(caps, nsb_limit=None):
    caps = np.asarray(caps)                     # [NSB, NQ, SB]

    nsb = int(nsb_limit or os.environ.get("K_NSB", NSB))
    S_total = int(caps.sum())
    T_total = S_total // P
    tiles_sb = caps.sum(axis=(1, 2)) // P       # tiles per superblock
    T_max = int(tiles_sb.max())
    OGRP = 8   # output row-pairs (128 rows each) staged per out DMA

    nc = bacc.Bacc("TRN2", target_bir_lowering=False, debug=False,
                   enable_asserts=False, num_devices=NCORES)
    xg_d = nc.dram_tensor("xg", [P, T_total, C], mybir.dt.float16,
                          kind="ExternalInput").ap()
    rv_d = nc.dram_tensor("rv", [P, 2 * T_total], mybir.dt.float32,
                          kind="ExternalInput").ap()
    ri_d = nc.dram_tensor("ri", [P, 2 * T_total], mybir.dt.int16,
                          kind="ExternalInput").ap()
    vh_d = nc.dram_tensor("vh", [P, 2 * T_total], mybir.dt.float16,
                          kind="ExternalInput").ap()
    xt_d = nc.dram_tensor("xt", [C, RPAD], mybir.dt.float32,
                          kind="ExternalInput").ap()
    w_d = nc.dram_tensor("w", [C, 2 * C], mybir.dt.float32,
                         kind="ExternalInput").ap()
    out_d = nc.dram_tensor("out", [128, RPAD // 128, C], mybir.dt.float32,
                           kind="ExternalOutput").ap()

    with tile.TileContext(nc) as tc:
        with tc.tile_pool(name="const", bufs=1) as cpool, \
             tc.tile_pool(name="gb", bufs=3) as gpool, \
             tc.tile_pool(name="meta", bufs=3) as mpool, \
             tc.tile_pool(name="oh", bufs=8) as ohpool, \
             tc.tile_pool(name="stg", bufs=2) as spool, \
             tc.tile_pool(name="ps1", bufs=2, space="PSUM") as ps1, \
             tc.tile_pool(name="ps2", bufs=2, space="PSUM") as ps2:

            # constants
            iota_i = cpool.tile([P, BP], mybir.dt.int16)
            nc.gpsimd.iota(iota_i[:], pattern=[[1, BP]], base=0,
                           channel_multiplier=0)
            iota_f = cpool.tile([P, BP], mybir.dt.float16)
            nc.vector.tensor_copy(iota_f[:], iota_i[:])
            w_t = cpool.tile([C, 2 * C], mybir.dt.float32)
            nc.sync.dma_start(w_t[:], w_d)
            xt_t = cpool.tile([C, RPAD], mybir.dt.float32)
            nc.sync.dma_start(xt_t[:], xt_d)

            reps = int(os.environ.get("K_REPS", "1"))
            import contextlib
            rep_ctx = tc.For_i(0, reps, 1) if reps > 1 else \
                contextlib.nullcontext()
            with rep_ctx:
                self_body(nc, tc, caps, nsb, tiles_sb, T_max, OGRP,
                          iota_f, w_t, xt_t, gpool, mpool, ohpool, spool,
                          ps1, ps2, xg_d, rv_d, ri_d, vh_d, out_d)
    nc.compile()
    return nc


def self_body(nc, tc, caps, nsb, tiles_sb, T_max, OGRP,
              iota_f, w_t, xt_t, gpool, mpool, ohpool, spool,
              ps1, ps2, xg_d, rv_d, ri_d, vh_d, out_d):
    runs_sb, _ = _assign(tiles_sb)
    tile_off = 0   # entry-tiles consumed so far
    ob = None

    PW = 128                         # output row-pair width
    NPAIR = RPAD // PW
    s2_wide = os.environ.get("K_S2", "block") == "wide"

    def stage2(s, psum_sT):
        # one wide PSUM->SBUF copy per superblock, then 128-row pair
        # weight matmuls + sigmoid + staged output DMA
        nonlocal ob
        k_sb = min(SB, NBLK - s * SB)
        rows = k_sb * BP
        sT_buf = spool.tile([C, SB * BP], mybir.dt.float32, tag="sT")
        if s2_wide:
            nc.scalar.copy(sT_buf[:, :rows], psum_sT[:, :rows])
        else:
            for j2 in range(rows // PW):
                nc.scalar.copy(sT_buf[:, j2 * PW:(j2 + 1) * PW],
                               psum_sT[:, j2 * PW:(j2 + 1) * PW])
        for j2 in range(rows // PW):
            b2 = (s * SB * BP) // PW + j2
            out2 = ps2.tile([PW, C], mybir.dt.float32)
            nc.tensor.matmul(out2[:], sT_buf[:, j2 * PW:(j2 + 1) * PW],
                             w_t[:, 0:C], start=True, stop=False)
            nc.tensor.matmul(out2[:], xt_t[:, b2 * PW:(b2 + 1) * PW],
                             w_t[:, C:2 * C], start=False, stop=True)

            g = b2 // OGRP
            j = b2 % OGRP
            gsz = min(OGRP, NPAIR - g * OGRP)
            if j == 0:
                ob = spool.tile([PW, OGRP, C], mybir.dt.float32,
                                tag="ob")
            nc.scalar.activation(ob[:, j, :], out2[:],
                                 mybir.ActivationFunctionType.Sigmoid)
            if j == gsz - 1:
                nc.sync.dma_start(
                    out_d[:, g * OGRP:g * OGRP + gsz, :],
                    ob[:, :gsz, :])

    pending = None   # (s, psum_sT) whose per-block stage runs one sb late
    for s in range(nsb):
        T_s = int(tiles_sb[s])

        rv_t = mpool.tile([P, 2 * T_max], mybir.dt.float32, tag="rv")
        nc.sync.dma_start(
            rv_t[:, :2 * T_s],
            rv_d[:, 2 * tile_off: 2 * (tile_off + T_s)])
        ri_t = mpool.tile([P, 2 * T_max], mybir.dt.int16, tag="ri")
        nc.sync.dma_start(
            ri_t[:, :2 * T_s],
            ri_d[:, 2 * tile_off: 2 * (tile_off + T_s)])
        vh_t = mpool.tile([P, 2 * T_max], mybir.dt.float16, tag="vh")
        nc.sync.dma_start(
            vh_t[:, :2 * T_s],
            vh_d[:, 2 * tile_off: 2 * (tile_off + T_s)])

        gbuf = gpool.tile([P, T_max, C], mybir.dt.float16, tag="g")
        nc.sync.dma_start(
            gbuf[:, :T_s, :],
            xg_d[:, tile_off:tile_off + T_s, :])

        # segment-sum all tiles into one PSUM bank [64, SB*BP]
        psum_sT = ps1.tile([C, SB * BP], mybir.dt.float32)
        tile_blocks = []
        for qq in range(NQ):
            for bb in range(SB):
                tile_blocks += [bb] * (int(caps[s, qq, bb]) // P)
        T_sb = len(tile_blocks)

        def mm(t, st_ap):
            # start=True zeroes the whole 2KB zero-region (= this
            # bank), initializing every block's 128-col span at once;
            # one accumulation group covers the whole superblock.
            nc.tensor.matmul(
                psum_sT[:, tile_blocks[t] * BP:(tile_blocks[t] + 1) * BP],
                gbuf[:, t, :], st_ap,
                start=(t == 0),
                stop=(t == T_sb - 1),
            )

        t = 0
        for kind, k in runs_sb[s]:
            if kind == "gp":
                # k-tile batched scaled one-hot via GPSIMD local scatter:
                # stb[:]=0; stb[p, rloc[p,j] + 128*j] = val[p,j]
                # (odd idxs are -1 -> ignored)
                stb = ohpool.tile([P, GPB * BP], mybir.dt.float16, tag="gob")
                nc.gpsimd.local_scatter(
                    stb[:, :k * BP],
                    vh_t[:, 2 * t:2 * (t + k)],
                    ri_t[:, 2 * t:2 * (t + k)],
                    channels=P, num_elems=k * BP, num_idxs=2 * k,
                )
                for j in range(k):
                    mm(t + j, stb[:, j * BP:(j + 1) * BP])
            else:
                for j in range(k):
                    st = ohpool.tile([P, BP], mybir.dt.float16, tag="oh")
                    nc.vector.tensor_scalar(
                        out=st[:],
                        in0=iota_f[:],
                        scalar1=rv_t[:, 2 * (t + j):2 * (t + j) + 1],
                        scalar2=rv_t[:, 2 * (t + j) + 1:2 * (t + j) + 2],
                        op0=mybir.AluOpType.is_equal,
                        op1=mybir.AluOpType.mult,
                    )
                    mm(t + j, st[:])
            t += k

        if pending is not None:
            stage2(*pending)
        pending = (s, psum_sT)

        tile_off += T_s

    if pending is not None:
        stage2(*pending)


# ---------------------------------------------------------------- entry

_CACHE = {}


def _prepare(inputs):
    in_maps, caps = _preprocess(inputs)
    key = caps.tobytes()
    if key not in _CACHE:
        _CACHE[key] = _build(caps)
    return _CACHE[key], in_maps


def kernel(**inputs):
    nc, in_maps = _prepare(inputs)
    res = bass_utils.run_bass_kernel_spmd(nc, in_maps,
                                          core_ids=list(range(NCORES)))
    outs = []
    for c in range(NCORES):
        o = res.results[c]["out"]          # [BP, NBLK, C]
        outs.append(o.transpose(1, 0, 2).reshape(RPAD, C)[:R])
    return np.concatenate(outs, axis=0).astype(np.float32)


# revision 28
# speedup vs baseline: 1.3223x; 1.0588x over previous
"""CANLayer (GNN message passing) Trainium2 kernel — 8 NeuronCores.

y = sigmoid(L_down @ (x Wc) + L_up @ (x Wc) + x Wl)

Strategy (self-contained: full inputs in, full output out):
  - segment_sum commutes with the dense right-multiplication by Wc, so we
    segment-sum raw x rows per 128-row destination block and apply Wc
    afterward:  s = segsum(val * x[col]);  y = sigmoid(s Wc + x Wl)
  - destination rows are sharded across 8 cores (12500 each).  Both
    Laplacians' COO entries are bucketed by (dest superblock of 4 blocks,
    source quarter, block) on the host; slot padding (caps = max count over
    cores, rounded to 128) keeps the instruction stream identical across
    cores so one SPMD program serves all 8.
  - the source-row gather x[col] is done ON THE HOST (pure permutation,
    np fancy-indexing, no arithmetic): per core a dense fp16 [128, T, 64]
    slot-ordered stream `xg` is staged in HBM and loaded with big
    line-rate HWDGE DMAs — no per-entry SWDGE descriptors.
  - per entry-tile of 128 rows, a scaled one-hot
    S^T[e, r] = val_e * (r == rloc_e) is built on DVE in one fused
    tensor_scalar (is_equal, mult), and the PE accumulates
    s^T[64, 128*SB] += G_t.T @ S_t^T into a single PSUM bank.
  - everything after the segment-sum is f32.
"""
import os

import numpy as np

import concourse.mybir as mybir
import concourse.tile as tile
from concourse import bacc
from concourse import bass_utils

N = 100000
C = 64
NCORES = 8
P = 128                    # entries per tile (matmul contraction dim)
BP = int(os.environ.get("K_BP", "64"))   # dest-block rows (one-hot width)
R = N // NCORES            # 12500 rows per core
NBLK = (R + BP - 1) // BP  # dest blocks per core
RPAD = NBLK * BP
NQ = 4
QROWS = N // NQ            # 25000
SB = 512 // BP             # dest blocks per superblock (one PSUM bank)
NSB = (NBLK + SB - 1) // SB


# one-hot producer assignment: per superblock, repeat [GPB gpsimd-batched
# tiles, DVB dve tiles].  Must match between host prep and device IR.
GPB = int(os.environ.get("K_GPB", "12"))
DVB = int(os.environ.get("K_DVB", "4"))


def _assign(tiles_sb):
    """Per sb: list of ('gp', k) / ('dve', k) runs + per-tile batch pos."""
    runs_sb = []
    pos_sb = []                                 # batch position per tile
    for T_s in tiles_sb:
        runs = []
        pos = []
        t = 0
        while t < int(T_s):
            k = min(GPB, int(T_s) - t)
            runs.append(("gp", k))
            pos.extend(range(k))
            t += k
            if t < int(T_s):
                k = min(DVB, int(T_s) - t)
                runs.append(("dve", k))
                pos.extend([0] * k)
                t += k
        runs_sb.append(runs)
        pos_sb.append(pos)
    return runs_sb, pos_sb


# ---------------------------------------------------------------- host prep

def _preprocess(inputs):
    x = np.ascontiguousarray(np.asarray(inputs["x"], dtype=np.float32))
    w_conv = np.asarray(inputs["w_conv"], dtype=np.float32)
    w_lin = np.asarray(inputs["w_lin"], dtype=np.float32)

    rows = np.concatenate([np.asarray(inputs["down_rows"]),
                           np.asarray(inputs["up_rows"])]).astype(np.int64)
    cols = np.concatenate([np.asarray(inputs["down_cols"]),
                           np.asarray(inputs["up_cols"])]).astype(np.int64)
    vals = np.concatenate([np.asarray(inputs["down_vals"]),
                           np.asarray(inputs["up_vals"])]).astype(np.float32)

    core = rows // R
    rl = rows % R
    blk = rl // BP
    rloc = rl - blk * BP
    q = cols // QROWS

    # group order: (core, superblock, quarter, block-in-superblock)
    sb = blk // SB
    bin_ = blk - sb * SB
    gkey = (sb * NQ + q) * SB + bin_            # within-core group id
    ngpc = NSB * NQ * SB                        # groups per core (incl ghosts)
    key = core * ngpc + gkey
    order = np.argsort(key, kind="stable")
    key_s = key[order]
    col_s = cols[order]
    rloc_s = rloc[order]
    vals_s = vals[order]

    ngroups = NCORES * ngpc
    counts = np.bincount(key_s, minlength=ngroups).reshape(NCORES, ngpc)
    caps = counts.max(axis=0)                   # [ngpc]
    # ghost groups (blocks beyond NBLK in the last superblock) stay size 0
    g_ids = np.arange(ngpc)
    g_blk = (g_ids // (NQ * SB)) * SB + (g_ids % SB)
    ghost = g_blk >= NBLK
    caps = np.where(ghost, 0, np.maximum(((caps + P - 1) // P) * P, P))
    S_total = int(caps.sum())
    T_total = S_total // P

    group_off = np.zeros(ngpc, dtype=np.int64)
    group_off[1:] = np.cumsum(caps)[:-1]

    starts = np.zeros(ngroups + 1, dtype=np.int64)
    starts[1:] = np.cumsum(counts.reshape(-1))
    within = np.arange(len(key_s)) - starts[key_s]
    slot = group_off[key_s % ngpc] + within
    ecore = key_s // ngpc

    col_pad = np.zeros((NCORES, S_total), dtype=np.int64)
    rloc_pad = np.zeros((NCORES, S_total), dtype=np.float32)
    val_pad = np.zeros((NCORES, S_total), dtype=np.float32)
    col_pad[ecore, slot] = col_s
    rloc_pad[ecore, slot] = rloc_s.astype(np.float32)
    val_pad[ecore, slot] = vals_s
    # padding slots keep col=0, rloc=0, val=0 -> zero contribution
    # NB: pad slots scatter val=0 to rloc=0 (harmless; duplicates with a
    # real rloc=0 entry in the same partition are fine for the DVE one-hot
    # but local_scatter forbids dup idxs -- it doesn't: dups are across
    # idx columns of one call, and we pass a single real idx per call.

    x16 = x.astype(np.float16)                  # [N, 64]
    wcwl = np.concatenate([w_conv, w_lin], axis=1)  # [64, 128] f32

    in_maps = []
    for c in range(NCORES):
        # host-side gather: dense slot-ordered source rows, [128, T, 64]
        xg = np.ascontiguousarray(
            x16[col_pad[c]].reshape(T_total, P, C).transpose(1, 0, 2))
        rv = np.empty((P, 2 * T_total), dtype=np.float32)
        rv[:, 0::2] = rloc_pad[c].reshape(T_total, P).T
        rv[:, 1::2] = val_pad[c].reshape(T_total, P).T
        # local_scatter metadata: idx pairs (rloc + 128*batch_pos, -1),
        # data pairs (val, 0); batch offsets follow the _assign pattern
        tiles_sb_h = caps.reshape(NSB, NQ * SB).sum(axis=1) // P
        _, pos_sb = _assign(tiles_sb_h)
        bpos = np.concatenate([np.asarray(p, dtype=np.int64)
                               for p in pos_sb if len(p)])
        ri = np.full((P, 2 * T_total), -1, dtype=np.int16)
        ri[:, 0::2] = (rloc_pad[c].reshape(T_total, P).T
                       + (bpos * BP)[None, :]).astype(np.int16)
        vh = np.zeros((P, 2 * T_total), dtype=np.float16)
        vh[:, 0::2] = val_pad[c].reshape(T_total, P).T.astype(np.float16)
        xT = np.zeros((C, RPAD), dtype=np.float32)
        xT[:, :R] = x[c * R:(c + 1) * R].T
        in_maps.append({
            "xg": xg,
            "rv": np.ascontiguousarray(rv),
            "ri": np.ascontiguousarray(ri),
            "vh": np.ascontiguousarray(vh),
            "xt": xT,
            "w": np.ascontiguousarray(wcwl),
        })
    return in_maps, caps.reshape(NSB, NQ, SB)


# ---------------------------------------------------------------- device IR

def _build(caps, nsb_limit=None):
    caps = np.asarray(caps)                     # [NSB, NQ, SB]

    nsb = int(nsb_limit or os.environ.get("K_NSB", NSB))
    S_total = int(caps.sum())
    T_total = S_total // P
    tiles_sb = caps.sum(axis=(1, 2)) // P       # tiles per superblock
    T_max = int(tiles_sb.max())
    OGRP = 8   # output row-pairs (128 rows each) staged per out DMA

    nc = bacc.Bacc("TRN2", target_bir_lowering=False, debug=False,
                   enable_asserts=False, num_devices=NCORES)
    xg_d = nc.dram_tensor("xg", [P, T_total, C], mybir.dt.float16,
                          kind="ExternalInput").ap()
    rv_d = nc.dram_tensor("rv", [P, 2 * T_total], mybir.dt.float32,
                          kind="ExternalInput").ap()
    ri_d = nc.dram_tensor("ri", [P, 2 * T_total], mybir.dt.int16,
                          kind="ExternalInput").ap()
    vh_d = nc.dram_tensor("vh", [P, 2 * T_total], mybir.dt.float16,
                          kind="ExternalInput").ap()
    xt_d = nc.dram_tensor("xt", [C, RPAD], mybir.dt.float32,
                          kind="ExternalInput").ap()
    w_d = nc.dram_tensor("w", [C, 2 * C], mybir.dt.float32,
                         kind="ExternalInput").ap()
    out_d = nc.dram_tensor("out", [128, RPAD // 128, C], mybir.dt.float32,
                           kind="ExternalOutput").ap()

    with tile.TileContext(nc) as tc:
        with tc.tile_pool(name="const", bufs=1) as cpool, \
             tc.tile_pool(name="gb", bufs=3) as gpool, \
             tc.tile_pool(name="meta", bufs=4) as mpool, \
             tc.tile_pool(name="oh", bufs=int(os.environ.get("K_OHB", "12"))) as ohpool, \
             tc.tile_pool(name="stg", bufs=2) as spool, \
             tc.tile_pool(name="ps1", bufs=2, space="PSUM") as ps1, \
             tc.tile_pool(name="ps2", bufs=int(os.environ.get("K_PS2", "4")), space="PSUM") as ps2:

            # constants
            iota_i = cpool.tile([P, BP], mybir.dt.int16)
            nc.gpsimd.iota(iota_i[:], pattern=[[1, BP]], base=0,
                           channel_multiplier=0)
            iota_f = cpool.tile([P, BP], mybir.dt.float16)
            nc.vector.tensor_copy(iota_f[:], iota_i[:])
            w_t = cpool.tile([C, 2 * C], mybir.dt.float32)
            nc.sync.dma_start(w_t[:], w_d)
            xt_t = cpool.tile([C, RPAD], mybir.dt.float32)
            nc.sync.dma_start(xt_t[:], xt_d)

            reps = int(os.environ.get("K_REPS", "1"))
            import contextlib
            rep_ctx = tc.For_i(0, reps, 1) if reps > 1 else \
                contextlib.nullcontext()
            with rep_ctx:
                self_body(nc, tc, caps, nsb, tiles_sb, T_max, OGRP,
                          iota_f, w_t, xt_t, gpool, mpool, ohpool, spool,
                          ps1, ps2, xg_d, rv_d, ri_d, vh_d, out_d)
    nc.compile()
    return nc


def self_body(nc, tc, caps, nsb, tiles_sb, T_max, OGRP,
              iota_f, w_t, xt_t, gpool, mpool, ohpool, spool,
              ps1, ps2, xg_d, rv_d, ri_d, vh_d, out_d):
    runs_sb, _ = _assign(tiles_sb)
    tile_off = 0   # entry-tiles consumed so far
    ob = None

    PW = 128                         # output row-pair width
    NPAIR = RPAD // PW
    s2_wide = os.environ.get("K_S2", "block") == "wide"

    def stage2(s, psum_sT):
        # one wide PSUM->SBUF copy per superblock, then 128-row pair
        # weight matmuls + sigmoid + staged output DMA
        nonlocal ob
        k_sb = min(SB, NBLK - s * SB)
        rows = k_sb * BP
        sT_buf = spool.tile([C, SB * BP], mybir.dt.float32, tag="sT")
        if s2_wide:
            nc.scalar.copy(sT_buf[:, :rows], psum_sT[:, :rows])
        else:
            for j2 in range(rows // PW):
                nc.scalar.copy(sT_buf[:, j2 * PW:(j2 + 1) * PW],
                               psum_sT[:, j2 * PW:(j2 + 1) * PW])
        for j2 in range(rows // PW):
            b2 = (s * SB * BP) // PW + j2
            out2 = ps2.tile([PW, C], mybir.dt.float32)
            nc.tensor.matmul(out2[:], sT_buf[:, j2 * PW:(j2 + 1) * PW],
                             w_t[:, 0:C], start=True, stop=False)
            nc.tensor.matmul(out2[:], xt_t[:, b2 * PW:(b2 + 1) * PW],
                             w_t[:, C:2 * C], start=False, stop=True)

            g = b2 // OGRP
            j = b2 % OGRP
            gsz = min(OGRP, NPAIR - g * OGRP)
            if j == 0:
                ob = spool.tile([PW, OGRP, C], mybir.dt.float32,
                                tag="ob")
            nc.scalar.activation(ob[:, j, :], out2[:],
                                 mybir.ActivationFunctionType.Sigmoid)
            if j == gsz - 1:
                nc.sync.dma_start(
                    out_d[:, g * OGRP:g * OGRP + gsz, :],
                    ob[:, :gsz, :])

    pending = None   # (s, psum_sT) whose per-block stage runs one sb late
    for s in range(nsb):
        T_s = int(tiles_sb[s])

        rv_t = mpool.tile([P, 2 * T_max], mybir.dt.float32, tag="rv")
        nc.sync.dma_start(
            rv_t[:, :2 * T_s],
            rv_d[:, 2 * tile_off: 2 * (tile_off + T_s)])
        ri_t = mpool.tile([P, 2 * T_max], mybir.dt.int16, tag="ri")
        nc.sync.dma_start(
            ri_t[:, :2 * T_s],
            ri_d[:, 2 * tile_off: 2 * (tile_off + T_s)])
        vh_t = mpool.tile([P, 2 * T_max], mybir.dt.float16, tag="vh")
        nc.sync.dma_start(
            vh_t[:, :2 * T_s],
            vh_d[:, 2 * tile_off: 2 * (tile_off + T_s)])

        gbuf = gpool.tile([P, T_max, C], mybir.dt.float16, tag="g")
        nc.sync.dma_start(
            gbuf[:, :T_s, :],
            xg_d[:, tile_off:tile_off + T_s, :])

        # segment-sum all tiles into one PSUM bank [64, SB*BP]
        psum_sT = ps1.tile([C, SB * BP], mybir.dt.float32)
        tile_blocks = []
        for qq in range(NQ):
            for bb in range(SB):
                tile_blocks += [bb] * (int(caps[s, qq, bb]) // P)
        T_sb = len(tile_blocks)

        def mm(t, st_ap):
            # start=True zeroes the whole 2KB zero-region (= this
            # bank), initializing every block's 128-col span at once;
            # one accumulation group covers the whole superblock.
            nc.tensor.matmul(
                psum_sT[:, tile_blocks[t] * BP:(tile_blocks[t] + 1) * BP],
                gbuf[:, t, :], st_ap,
                start=(t == 0),
                stop=(t == T_sb - 1),
            )

        t = 0
        for kind, k in runs_sb[s]:
            if kind == "gp":
                # k-tile batched scaled one-hot via GPSIMD local scatter:
                # stb[:]=0; stb[p, rloc[p,j] + 128*j] = val[p,j]
                # (odd idxs are -1 -> ignored)
                stb = ohpool.tile([P, GPB * BP], mybir.dt.float16, tag="gob")
                nc.gpsimd.local_scatter(
                    stb[:, :k * BP],
                    vh_t[:, 2 * t:2 * (t + k)],
                    ri_t[:, 2 * t:2 * (t + k)],
                    channels=P, num_elems=k * BP, num_idxs=2 * k,
                )
                for j in range(k):
                    mm(t + j, stb[:, j * BP:(j + 1) * BP])
            else:
                for j in range(k):
                    st = ohpool.tile([P, BP], mybir.dt.float16, tag="oh")
                    nc.vector.tensor_scalar(
                        out=st[:],
                        in0=iota_f[:],
                        scalar1=rv_t[:, 2 * (t + j):2 * (t + j) + 1],
                        scalar2=rv_t[:, 2 * (t + j) + 1:2 * (t + j) + 2],
                        op0=mybir.AluOpType.is_equal,
                        op1=mybir.AluOpType.mult,
                    )
                    mm(t + j, st[:])
            t += k

        if pending is not None:
            stage2(*pending)
        pending = (s, psum_sT)

        tile_off += T_s

    if pending is not None:
        stage2(*pending)


# ---------------------------------------------------------------- entry

_CACHE = {}


def _prepare(inputs):
    in_maps, caps = _preprocess(inputs)
    key = caps.tobytes()
    if key not in _CACHE:
        _CACHE[key] = _build(caps)
    return _CACHE[key], in_maps


def kernel(**inputs):
    nc, in_maps = _prepare(inputs)
    res = bass_utils.run_bass_kernel_spmd(nc, in_maps,
                                          core_ids=list(range(NCORES)))
    outs = []
    for c in range(NCORES):
        o = res.results[c]["out"]          # [BP, NBLK, C]
        outs.append(o.transpose(1, 0, 2).reshape(RPAD, C)[:R])
    return np.concatenate(outs, axis=0).astype(np.float32)


# revision 29
# speedup vs baseline: 1.3465x; 1.0183x over previous
"""CANLayer (GNN message passing) Trainium2 kernel — 8 NeuronCores.

y = sigmoid(L_down @ (x Wc) + L_up @ (x Wc) + x Wl)

Strategy (self-contained: full inputs in, full output out):
  - segment_sum commutes with the dense right-multiplication by Wc, so we
    segment-sum raw x rows per 128-row destination block and apply Wc
    afterward:  s = segsum(val * x[col]);  y = sigmoid(s Wc + x Wl)
  - destination rows are sharded across 8 cores (12500 each).  Both
    Laplacians' COO entries are bucketed by (dest superblock of 4 blocks,
    source quarter, block) on the host; slot padding (caps = max count over
    cores, rounded to 128) keeps the instruction stream identical across
    cores so one SPMD program serves all 8.
  - the source-row gather x[col] is done ON THE HOST (pure permutation,
    np fancy-indexing, no arithmetic): per core a dense fp16 [128, T, 64]
    slot-ordered stream `xg` is staged in HBM and loaded with big
    line-rate HWDGE DMAs — no per-entry SWDGE descriptors (the on-device
    dma_gather path was Q7 descriptor-generation bound at ~30ns/idx).
  - per entry-tile of 128 rows, a scaled one-hot
    S^T[e, r] = val_e * (r == rloc_e) feeds the PE, which accumulates
    s^T[64, SB*BP] += G_t.T @ S_t^T into a single PSUM bank.  One-hot
    construction is split across two producers rate-matched ~3:1 —
    GPSIMD LocalScatter (k-tile batches: one op zeroes [128, k*BP] and
    scatters val at rloc + BP*j; ~75ns/tile) and DVE fused tensor_scalar
    (is_equal, mult; ~215ns/tile) — so DVE, GpSimd, PE and the DMA
    engines all run ~70% busy in parallel.
  - stage 2 (one sb behind accumulation): per-128-col ACT copies
    PSUM->SBUF (a single bank-wide 512-col copy races on HW — keep the
    copies <=128 cols), then per 128-row pair one Wc and one Wl matmul
    accumulate in PSUM and ACT applies the sigmoid; everything after
    the segment-sum is f32.
"""
import os

import numpy as np

import concourse.mybir as mybir
import concourse.tile as tile
from concourse import bacc
from concourse import bass_utils

N = 100000
C = 64
NCORES = 8
P = 128                    # entries per tile (matmul contraction dim)
BP = int(os.environ.get("K_BP", "64"))   # dest-block rows (one-hot width)
R = N // NCORES            # 12500 rows per core
NBLK = (R + BP - 1) // BP  # dest blocks per core
RPAD = NBLK * BP
NQ = 4
QROWS = N // NQ            # 25000
SB = 512 // BP             # dest blocks per superblock (one PSUM bank)
NSB = (NBLK + SB - 1) // SB


# one-hot producer assignment: per superblock, repeat [GPB gpsimd-batched
# tiles, DVB dve tiles].  Must match between host prep and device IR.
GPB = int(os.environ.get("K_GPB", "12"))
DVB = int(os.environ.get("K_DVB", "4"))


def _assign(tiles_sb):
    """Per sb: list of ('gp', k) / ('dve', k) runs + per-tile batch pos."""
    runs_sb = []
    pos_sb = []                                 # batch position per tile
    for T_s in tiles_sb:
        runs = []
        pos = []
        t = 0
        while t < int(T_s):
            k = min(GPB, int(T_s) - t)
            runs.append(("gp", k))
            pos.extend(range(k))
            t += k
            if t < int(T_s):
                k = min(DVB, int(T_s) - t)
                runs.append(("dve", k))
                pos.extend([0] * k)
                t += k
        runs_sb.append(runs)
        pos_sb.append(pos)
    return runs_sb, pos_sb


# ---------------------------------------------------------------- host prep

def _preprocess(inputs):
    x = np.ascontiguousarray(np.asarray(inputs["x"], dtype=np.float32))
    w_conv = np.asarray(inputs["w_conv"], dtype=np.float32)
    w_lin = np.asarray(inputs["w_lin"], dtype=np.float32)

    rows = np.concatenate([np.asarray(inputs["down_rows"]),
                           np.asarray(inputs["up_rows"])]).astype(np.int64)
    cols = np.concatenate([np.asarray(inputs["down_cols"]),
                           np.asarray(inputs["up_cols"])]).astype(np.int64)
    vals = np.concatenate([np.asarray(inputs["down_vals"]),
                           np.asarray(inputs["up_vals"])]).astype(np.float32)

    core = rows // R
    rl = rows % R
    blk = rl // BP
    rloc = rl - blk * BP
    q = cols // QROWS

    # group order: (core, superblock, quarter, block-in-superblock)
    sb = blk // SB
    bin_ = blk - sb * SB
    gkey = (sb * NQ + q) * SB + bin_            # within-core group id
    ngpc = NSB * NQ * SB                        # groups per core (incl ghosts)
    key = core * ngpc + gkey
    order = np.argsort(key, kind="stable")
    key_s = key[order]
    col_s = cols[order]
    rloc_s = rloc[order]
    vals_s = vals[order]

    ngroups = NCORES * ngpc
    counts = np.bincount(key_s, minlength=ngroups).reshape(NCORES, ngpc)
    caps = counts.max(axis=0)                   # [ngpc]
    # ghost groups (blocks beyond NBLK in the last superblock) stay size 0
    g_ids = np.arange(ngpc)
    g_blk = (g_ids // (NQ * SB)) * SB + (g_ids % SB)
    ghost = g_blk >= NBLK
    caps = np.where(ghost, 0, np.maximum(((caps + P - 1) // P) * P, P))
    S_total = int(caps.sum())
    T_total = S_total // P

    group_off = np.zeros(ngpc, dtype=np.int64)
    group_off[1:] = np.cumsum(caps)[:-1]

    starts = np.zeros(ngroups + 1, dtype=np.int64)
    starts[1:] = np.cumsum(counts.reshape(-1))
    within = np.arange(len(key_s)) - starts[key_s]
    slot = group_off[key_s % ngpc] + within
    ecore = key_s // ngpc

    col_pad = np.zeros((NCORES, S_total), dtype=np.int64)
    rloc_pad = np.zeros((NCORES, S_total), dtype=np.float32)
    val_pad = np.zeros((NCORES, S_total), dtype=np.float32)
    col_pad[ecore, slot] = col_s
    rloc_pad[ecore, slot] = rloc_s.astype(np.float32)
    val_pad[ecore, slot] = vals_s
    # padding slots keep col=0, rloc=0, val=0 -> zero contribution
    # NB: pad slots scatter val=0 to rloc=0 (harmless; duplicates with a
    # real rloc=0 entry in the same partition are fine for the DVE one-hot
    # but local_scatter forbids dup idxs -- it doesn't: dups are across
    # idx columns of one call, and we pass a single real idx per call.

    x16 = x.astype(np.float16)                  # [N, 64]
    wcwl = np.concatenate([w_conv, w_lin], axis=1)  # [64, 128] f32

    in_maps = []
    for c in range(NCORES):
        # host-side gather: dense slot-ordered source rows, [128, T, 64]
        xg = np.ascontiguousarray(
            x16[col_pad[c]].reshape(T_total, P, C).transpose(1, 0, 2))
        rv = np.empty((P, 2 * T_total), dtype=np.float32)
        rv[:, 0::2] = rloc_pad[c].reshape(T_total, P).T
        rv[:, 1::2] = val_pad[c].reshape(T_total, P).T
        # local_scatter metadata: idx pairs (rloc + 128*batch_pos, -1),
        # data pairs (val, 0); batch offsets follow the _assign pattern
        tiles_sb_h = caps.reshape(NSB, NQ * SB).sum(axis=1) // P
        _, pos_sb = _assign(tiles_sb_h)
        bpos = np.concatenate([np.asarray(p, dtype=np.int64)
                               for p in pos_sb if len(p)])
        ri = np.full((P, 2 * T_total), -1, dtype=np.int16)
        ri[:, 0::2] = (rloc_pad[c].reshape(T_total, P).T
                       + (bpos * BP)[None, :]).astype(np.int16)
        vh = np.zeros((P, 2 * T_total), dtype=np.float16)
        vh[:, 0::2] = val_pad[c].reshape(T_total, P).T.astype(np.float16)
        xT = np.zeros((C, RPAD), dtype=np.float32)
        xT[:, :R] = x[c * R:(c + 1) * R].T
        in_maps.append({
            "xg": xg,
            "rv": np.ascontiguousarray(rv),
            "ri": np.ascontiguousarray(ri),
            "vh": np.ascontiguousarray(vh),
            "xt": xT,
            "w": np.ascontiguousarray(wcwl),
        })
    return in_maps, caps.reshape(NSB, NQ, SB)


# ---------------------------------------------------------------- device IR

def _build(caps, nsb_limit=None):
    caps = np.asarray(caps)                     # [NSB, NQ, SB]

    nsb = int(nsb_limit or os.environ.get("K_NSB", NSB))
    S_total = int(caps.sum())
    T_total = S_total // P
    tiles_sb = caps.sum(axis=(1, 2)) // P       # tiles per superblock
    T_max = int(tiles_sb.max())
    OGRP = 8   # output row-pairs (128 rows each) staged per out DMA

    nc = bacc.Bacc("TRN2", target_bir_lowering=False, debug=False,
                   enable_asserts=False, num_devices=NCORES)
    xg_d = nc.dram_tensor("xg", [P, T_total, C], mybir.dt.float16,
                          kind="ExternalInput").ap()
    rv_d = nc.dram_tensor("rv", [P, 2 * T_total], mybir.dt.float32,
                          kind="ExternalInput").ap()
    ri_d = nc.dram_tensor("ri", [P, 2 * T_total], mybir.dt.int16,
                          kind="ExternalInput").ap()
    vh_d = nc.dram_tensor("vh", [P, 2 * T_total], mybir.dt.float16,
                          kind="ExternalInput").ap()
    xt_d = nc.dram_tensor("xt", [C, RPAD], mybir.dt.float32,
                          kind="ExternalInput").ap()
    w_d = nc.dram_tensor("w", [C, 2 * C], mybir.dt.float32,
                         kind="ExternalInput").ap()
    out_d = nc.dram_tensor("out", [128, RPAD // 128, C], mybir.dt.float32,
                           kind="ExternalOutput").ap()

    with tile.TileContext(nc) as tc:
        with tc.tile_pool(name="const", bufs=1) as cpool, \
             tc.tile_pool(name="gb", bufs=3) as gpool, \
             tc.tile_pool(name="meta", bufs=4) as mpool, \
             tc.tile_pool(name="oh", bufs=int(os.environ.get("K_OHB", "12"))) as ohpool, \
             tc.tile_pool(name="stg", bufs=2) as spool, \
             tc.tile_pool(name="ps1", bufs=2, space="PSUM") as ps1, \
             tc.tile_pool(name="ps2", bufs=int(os.environ.get("K_PS2", "4")), space="PSUM") as ps2:

            # constants
            iota_i = cpool.tile([P, BP], mybir.dt.int16)
            nc.gpsimd.iota(iota_i[:], pattern=[[1, BP]], base=0,
                           channel_multiplier=0)
            iota_f = cpool.tile([P, BP], mybir.dt.float16)
            nc.vector.tensor_copy(iota_f[:], iota_i[:])
            w_t = cpool.tile([C, 2 * C], mybir.dt.float32)
            nc.sync.dma_start(w_t[:], w_d)
            xt_t = cpool.tile([C, RPAD], mybir.dt.float32)
            nc.sync.dma_start(xt_t[:], xt_d)

            reps = int(os.environ.get("K_REPS", "1"))
            import contextlib
            rep_ctx = tc.For_i(0, reps, 1) if reps > 1 else \
                contextlib.nullcontext()
            with rep_ctx:
                self_body(nc, tc, caps, nsb, tiles_sb, T_max, OGRP,
                          iota_f, w_t, xt_t, gpool, mpool, ohpool, spool,
                          ps1, ps2, xg_d, rv_d, ri_d, vh_d, out_d)
    nc.compile()
    return nc


def self_body(nc, tc, caps, nsb, tiles_sb, T_max, OGRP,
              iota_f, w_t, xt_t, gpool, mpool, ohpool, spool,
              ps1, ps2, xg_d, rv_d, ri_d, vh_d, out_d):
    runs_sb, _ = _assign(tiles_sb)
    tile_off = 0   # entry-tiles consumed so far
    ob = None

    PW = 128                         # output row-pair width
    NPAIR = RPAD // PW
    s2_wide = os.environ.get("K_S2", "block") == "wide"

    def stage2(s, psum_sT):
        # one wide PSUM->SBUF copy per superblock, then 128-row pair
        # weight matmuls + sigmoid + staged output DMA
        nonlocal ob
        k_sb = min(SB, NBLK - s * SB)
        rows = k_sb * BP
        sT_buf = spool.tile([C, SB * BP], mybir.dt.float32, tag="sT")
        if s2_wide:
            nc.scalar.copy(sT_buf[:, :rows], psum_sT[:, :rows])
        else:
            for j2 in range(rows // PW):
                nc.scalar.copy(sT_buf[:, j2 * PW:(j2 + 1) * PW],
                               psum_sT[:, j2 * PW:(j2 + 1) * PW])
        for j2 in range(rows // PW):
            b2 = (s * SB * BP) // PW + j2
            out2 = ps2.tile([PW, C], mybir.dt.float32)
            nc.tensor.matmul(out2[:], sT_buf[:, j2 * PW:(j2 + 1) * PW],
                             w_t[:, 0:C], start=True, stop=False)
            nc.tensor.matmul(out2[:], xt_t[:, b2 * PW:(b2 + 1) * PW],
                             w_t[:, C:2 * C], start=False, stop=True)

            g = b2 // OGRP
            j = b2 % OGRP
            gsz = min(OGRP, NPAIR - g * OGRP)
            if j == 0:
                ob = spool.tile([PW, OGRP, C], mybir.dt.float32,
                                tag="ob")
            nc.scalar.activation(ob[:, j, :], out2[:],
                                 mybir.ActivationFunctionType.Sigmoid)
            if j == gsz - 1:
                nc.sync.dma_start(
                    out_d[:, g * OGRP:g * OGRP + gsz, :],
                    ob[:, :gsz, :])

    pending = None   # (s, psum_sT) whose per-block stage runs one sb late
    for s in range(nsb):
        T_s = int(tiles_sb[s])

        rv_t = mpool.tile([P, 2 * T_max], mybir.dt.float32, tag="rv")
        nc.sync.dma_start(
            rv_t[:, :2 * T_s],
            rv_d[:, 2 * tile_off: 2 * (tile_off + T_s)])
        ri_t = mpool.tile([P, 2 * T_max], mybir.dt.int16, tag="ri")
        nc.sync.dma_start(
            ri_t[:, :2 * T_s],
            ri_d[:, 2 * tile_off: 2 * (tile_off + T_s)])
        vh_t = mpool.tile([P, 2 * T_max], mybir.dt.float16, tag="vh")
        nc.sync.dma_start(
            vh_t[:, :2 * T_s],
            vh_d[:, 2 * tile_off: 2 * (tile_off + T_s)])

        gbuf = gpool.tile([P, T_max, C], mybir.dt.float16, tag="g")
        nc.sync.dma_start(
            gbuf[:, :T_s, :],
            xg_d[:, tile_off:tile_off + T_s, :])

        # segment-sum all tiles into one PSUM bank [64, SB*BP]
        psum_sT = ps1.tile([C, SB * BP], mybir.dt.float32)
        tile_blocks = []
        for qq in range(NQ):
            for bb in range(SB):
                tile_blocks += [bb] * (int(caps[s, qq, bb]) // P)
        T_sb = len(tile_blocks)

        def mm(t, st_ap):
            # start=True zeroes the whole 2KB zero-region (= this
            # bank), initializing every block's 128-col span at once;
            # one accumulation group covers the whole superblock.
            nc.tensor.matmul(
                psum_sT[:, tile_blocks[t] * BP:(tile_blocks[t] + 1) * BP],
                gbuf[:, t, :], st_ap,
                start=(t == 0),
                stop=(t == T_sb - 1),
            )

        t = 0
        for kind, k in runs_sb[s]:
            if kind == "gp":
                # k-tile batched scaled one-hot via GPSIMD local scatter:
                # stb[:]=0; stb[p, rloc[p,j] + 128*j] = val[p,j]
                # (odd idxs are -1 -> ignored)
                stb = ohpool.tile([P, GPB * BP], mybir.dt.float16, tag="gob")
                nc.gpsimd.local_scatter(
                    stb[:, :k * BP],
                    vh_t[:, 2 * t:2 * (t + k)],
                    ri_t[:, 2 * t:2 * (t + k)],
                    channels=P, num_elems=k * BP, num_idxs=2 * k,
                )
                for j in range(k):
                    mm(t + j, stb[:, j * BP:(j + 1) * BP])
            else:
                for j in range(k):
                    st = ohpool.tile([P, BP], mybir.dt.float16, tag="oh")
                    nc.vector.tensor_scalar(
                        out=st[:],
                        in0=iota_f[:],
                        scalar1=rv_t[:, 2 * (t + j):2 * (t + j) + 1],
                        scalar2=rv_t[:, 2 * (t + j) + 1:2 * (t + j) + 2],
                        op0=mybir.AluOpType.is_equal,
                        op1=mybir.AluOpType.mult,
                    )
                    mm(t + j, st[:])
            t += k

        if pending is not None:
            stage2(*pending)
        pending = (s, psum_sT)

        tile_off += T_s

    if pending is not None:
        stage2(*pending)


# ---------------------------------------------------------------- entry

_CACHE = {}


def _prepare(inputs):
    in_maps, caps = _preprocess(inputs)
    key = caps.tobytes()
    if key not in _CACHE:
        _CACHE[key] = _build(caps)
    return _CACHE[key], in_maps


def kernel(**inputs):
    nc, in_maps = _prepare(inputs)
    res = bass_utils.run_bass_kernel_spmd(nc, in_maps,
                                          core_ids=list(range(NCORES)))
    outs = []
    for c in range(NCORES):
        o = res.results[c]["out"]          # [BP, NBLK, C]
        outs.append(o.transpose(1, 0, 2).reshape(RPAD, C)[:R])
    return np.concatenate(outs, axis=0).astype(np.float32)
